# revision 100
# baseline (speedup 1.0000x reference)
"""Trainium2 Bass kernel for nn_ArcDecoderLayer (sparse_attention).

Self-contained: takes FULL unsharded inputs, shards across 8 NeuronCores
(head-parallel attention, row-parallel o_proj with AllReduce, FF-parallel
MLP with AllGather of the intermediate), returns the FULL output.

v2 layout/schedule:
- LN1/LN2 statistics as concurrent col-tiled (sum, sumsq) matmul pairs;
  raw sums broadcast with 1/D folded into the broadcast constant so all
  stats math runs partition-aligned on full tiles.
- Attention: per-key-tile waves; score MMs row-tiled concurrent pairs,
  AV MMs col-tiled concurrent pairs, denominator MMs concurrent pairs,
  software-pipelined one key-tile ahead of the ScalarE exp; diagonal
  band tiles narrowed to their unmasked column range; packed full-width
  RoPE.
- o_proj computed locally from this core's heads into a full-D partial;
  per-block ReduceScatter(+AllGather) gives the residual rows and the
  full h sum (no attn/o AllGathers).
- MLP: gate/up per 512-block; row-parallel down-proj straight from the
  SBUF-resident m slice into a full-D partial, ReduceScattered back (no
  m AllGather, no 33MB of gathered-m reads).
All matmul compute bf16 with f32 PSUM accumulation.
"""

import sys
import types

sys.path.insert(0, "/opt/trn_rl_repo")

# ---- shim antenv.axon_hooks so trace=True profiling works in this image ----
if "antenv.axon_hooks" not in sys.modules:
    _hook_mod = types.ModuleType("antenv.axon_hooks")
    _hook_state = {"hook": None}

    def _set_hook(h):
        _hook_state["hook"] = h

    def _get_hook():
        return _hook_state["hook"]

    _hook_mod.set_axon_ntff_profile_hook = _set_hook
    _hook_mod.get_axon_ntff_profile_hook = _get_hook
    sys.modules["antenv.axon_hooks"] = _hook_mod
    try:
        import antenv

        antenv.axon_hooks = _hook_mod
        from trn_agent_boot.trn_boot import _ntff_profile_via_ctypes

        _set_hook(_ntff_profile_via_ctypes("/opt/axon/libaxon_pjrt.so"))
    except Exception:
        pass

import numpy as np
import ml_dtypes

import concourse.bass as bass
import concourse.mybir as mybir
import concourse.tile as tile
from concourse import library_config
from concourse.vector_clock import ScopedClock

BF16 = ml_dtypes.bfloat16

N_CORES = 8
D = 2048
FF = 8192
H = 32
DH = 64
RD = 16
EPS = 1e-5
BASE = 10000.0

J = D // N_CORES        # 256 head-dims per core (4 heads)
FFL = FF // N_CORES     # 1024 ff dims per core
KC = D // 128           # 16 contraction chunks
NBLK = 512              # lq block width
MD = J // 128           # 2 output Mtiles per core for down/out


WAIT_LIMITS = {"InstNoOp": 1, "InstDrain": 1, "InstEventSemaphore": 1}
DEFAULT_WAIT_LIMIT = 1


class PatchedTC(tile.TileContext):
    """TileContext patched for this walrus build, which rejects instructions
    carrying more than a couple of sync wait commands: excess waits are
    split onto injected same-engine nops just before the instruction."""

    _wsplit_n = 0

    def _split_excess_waits(self, ordered):
        for bb, insts in ordered.items():
            out = []
            for inst in insts:
                si = inst.sync_info
                waits = list(si.on_wait) if si and si.on_wait else []
                lim = WAIT_LIMITS.get(type(inst).__name__,
                                      DEFAULT_WAIT_LIMIT)
                if len(waits) > lim:
                    for w in waits[:-lim]:
                        nop = mybir.InstNoOp(
                            name=f"I-wsplit-{PatchedTC._wsplit_n}",
                            ins=[], outs=[], engine=inst.engine,
                            nofuse=True)
                        PatchedTC._wsplit_n += 1
                        nop.sync_info = mybir.SyncInfo(
                            on_wait=[w], on_update=[])
                        out.append(nop)
                    inst.sync_info = mybir.SyncInfo(
                        on_wait=waits[-lim:],
                        on_update=list(si.on_update or []))
                out.append(inst)
            ordered[bb] = out

    def _lower_ordered_insts(self, ordered):
        self._split_excess_waits(ordered)
        return super()._lower_ordered_insts(ordered)

    def _drain_and_barrier(self, tick_clock, wait_clock):
        nc = self.nc
        probe = nc.sync.nop(nofuse=True, hint="tail_wait_probe")
        wait_clock.add_sem_waits(
            probe.ins, ScopedClock({None: tick_clock.global_clock})
        )
        waits = list(probe.ins.sync_info.on_wait or [])
        probe.ins.sync_info.on_wait = waits[:1]
        for i in range(1, len(waits)):
            n = nc.sync.nop(nofuse=True, hint=f"tail_wait_{i}")
            n.ins.sync_info = mybir.SyncInfo(on_wait=[waits[i]], on_update=[])
        nc.sync.drain()
        nc.all_engine_barrier()
        assert self.sems is not None
        popped = nc._tile_sem_poison_stack.pop()
        assert popped is self._sem_poison
        nc.clear_and_free_semaphores(list(self.sems.allocated().values()))
        nc.all_engine_barrier()


def build_graph(S):
    """Build the SPMD 8-core graph for sequence length S (multiple of 512)."""
    dt = mybir.dt
    f32, bf16 = dt.float32, dt.bfloat16
    AF = mybir.ActivationFunctionType
    Alu = mybir.AluOpType
    NB = S // NBLK          # lq blocks (4)
    LT = S // 128           # 128-wide l tiles per part
    S2 = 2 * S

    nc = bass.Bass()
    P = nc.declare_dram_parameter

    xm_e = P("xm", [128, KC, S], bf16, isOutput=False)
    xh_e = P("xh", [128, KC, S], bf16, isOutput=False)
    xhres_e = P("xh_res", [128, MD, S], bf16, isOutput=False)
    wq_e = P("wq", [128, KC, J], bf16, isOutput=False)
    wk_e = P("wk", [128, KC, J], bf16, isOutput=False)
    wv_e = P("wv", [128, KC, J], bf16, isOutput=False)
    wo_e = P("wo_p", [128, MD, D], bf16, isOutput=False)
    wg_e = P("wg", [128, KC, FFL], bf16, isOutput=False)
    wu_e = P("wu", [128, KC, FFL], bf16, isOutput=False)
    wd_e = P("wd", [128, FFL // 128, D], bf16, isOutput=False)
    # column (per-partition) weight rowsums + biases for q/k/vTh epilogues
    wsq_e = P("wsq", [128, 2], f32, isOutput=False)
    wsk_e = P("wsk", [128, 2], f32, isOutput=False)
    wsvc_e = P("wsvc", [128, 2], f32, isOutput=False)   # for vT_h epilogue
    bq_e = P("bq", [128, 2], f32, isOutput=False)
    bk_e = P("bk", [128, 2], f32, isOutput=False)
    bvc_e = P("bvc", [128, 2], f32, isOutput=False)
    # row layouts for v_mem epilogue
    wsv_e = P("wsv_row", [1, J], f32, isOutput=False)
    bv_e = P("bv_row", [1, J], f32, isOutput=False)
    bg_e = P("bg", [128, FFL // 128], f32, isOutput=False)
    bu_e = P("bu", [128, FFL // 128], f32, isOutput=False)
    ropec_e = P("rope_cos", [128, S2], bf16, isOutput=False)
    ropes_e = P("rope_sinsg", [128, S2], bf16, isOutput=False)
    masks_e = P("masks", [128, 4, NBLK], bf16, isOutput=False)
    out_e = P("out", [MD, 128, S], f32, isOutput=True)

    rg = [list(range(N_CORES))]

    # collective groupings
    ARG = [(b, b + 1) for b in range(NB)]   # o-sum RS+AG: per block
    DRG = [(0, 2), (2, 3), (3, NB)]         # down-partial RS groups

    def argrp(b):
        for g, (lo, hi) in enumerate(ARG):
            if lo <= b < hi:
                return g, lo, hi

    def drgrp(b):
        for g, (lo, hi) in enumerate(DRG):
            if lo <= b < hi:
                return g, lo, hi

    with PatchedTC(nc) as tc:
        with (
            tc.tile_pool(name="const", bufs=1) as constp,
            tc.tile_pool(name="dram", bufs=1, space="DRAM") as dramp,
            tc.tile_pool(name="dsh", bufs=1, space="DRAM") as dshp,
        ):
            kqvp = tc.alloc_tile_pool(name="kqv", bufs=1)
            statkp = tc.alloc_tile_pool(name="statk", bufs=1)
            ones_c = constp.tile([128, 1], bf16)
            nc.vector.memset(ones_c[:], 1.0)
            ones128 = constp.tile([128, 128], bf16)
            nc.vector.memset(ones128[:], 1.0)
            invD128 = constp.tile([128, 128], bf16)
            nc.vector.memset(invD128[:], 1.0 / D)
            eps_c = constp.tile([128, 1], f32)
            nc.vector.memset(eps_c[:], EPS)
            onesf = constp.tile([1, 128], f32)
            nc.vector.memset(onesf[:], 1.0)

            def bcast_rows(dst, src_row, width, pspool, ones_row):
                """dst[0:128, :width] = src_row[0, :width] via K=1 matmuls
                (partition_broadcast is not encodable by this walrus)."""
                for i in range(0, width, NBLK):
                    w = min(NBLK, width - i)
                    ps = pspool.tile([128, NBLK], f32, name="bc_ps",
                                     tag="bc_ps", bufs=1)
                    nc.tensor.matmul(ps[:, :w], ones_row[0:1, :],
                                     src_row[0:1, i:i + w],
                                     start=True, stop=True)
                    nc.vector.tensor_copy(dst[:, i:i + w], ps[:, :w])

            wsvb = constp.tile([128, J], f32)
            wsv_row = constp.tile([1, J], f32)
            nc.sync.dma_start(wsv_row[:], wsv_e[:])
            bvb = constp.tile([128, J], f32)
            bv_row = constp.tile([1, J], f32)
            nc.sync.dma_start(bv_row[:], bv_e[:])
            wsq_t = constp.tile([128, 2], f32)
            nc.sync.dma_start(wsq_t[:], wsq_e[:])
            wsk_t = constp.tile([128, 2], f32)
            nc.sync.dma_start(wsk_t[:], wsk_e[:])
            wsvc_t = constp.tile([128, 2], f32)
            nc.sync.dma_start(wsvc_t[:], wsvc_e[:])
            bq_t = constp.tile([128, 2], f32)
            nc.sync.dma_start(bq_t[:], bq_e[:])
            bk_t = constp.tile([128, 2], f32)
            nc.sync.dma_start(bk_t[:], bk_e[:])
            bvc_t = constp.tile([128, 2], f32)
            nc.sync.dma_start(bvc_t[:], bvc_e[:])
            bg_t = constp.tile([128, FFL // 128], f32)
            nc.sync.dma_start(bg_t[:], bg_e[:])
            bu_t = constp.tile([128, FFL // 128], f32)
            nc.sync.dma_start(bu_t[:], bu_e[:])

            # persistent QKV outputs
            kT = [kqvp.tile([128, S2], bf16, name=f"kT{m}") for m in range(2)]
            qT = [kqvp.tile([128, S], bf16, name=f"qT{m}") for m in range(2)]
            vTh = [kqvp.tile([128, S], bf16, name=f"vTh{m}") for m in range(2)]
            v_mem = kqvp.tile([128, LT, J], bf16)

            # v_mem epilogue needs column-layout stats of the mem part
            rstd_col_mem = statkp.tile([128, LT], f32)
            c_col_mem = statkp.tile([128, LT], f32)

            # rope tables (persistent through phase 1)
            ropep = tc.alloc_tile_pool(name="rope", bufs=1)
            cos_t = ropep.tile([128, S], bf16)
            sin_t = ropep.tile([128, S], bf16)
            rope_loaded = [False]

            def load_rope_tables():
                if not rope_loaded[0]:
                    nc.sync.dma_start(cos_t[:], ropec_e[:, 0:S])
                    nc.sync.dma_start(sin_t[:], ropes_e[:, 0:S])
                    rope_loaded[0] = True

            def rope_packed(slices, name):
                """Apply partial rotary to up to 8 (tile, col_lo) 16-row
                rotary groups at once, packed across all 128 partitions.
                Each slice is (tile, row_lo, col_lo); processes
                tile[row_lo:row_lo+16, col_lo:col_lo+S]. The cos/sin
                tables have the same 16-row pattern on every group, and
                positions repeat across both S-halves."""
                pk = ropep.tile([128, S], bf16, name=f"pk_{name}",
                                tag="rope_pk", bufs=2)
                sw = ropep.tile([128, S], bf16, name=f"sw_{name}",
                                tag="rope_sw", bufs=2)
                for i, (t, rlo, clo) in enumerate(slices):
                    csl = slice(clo, clo + S)
                    eng = nc.sync if i % 2 == 0 else nc.scalar
                    eng.dma_start(pk[16 * i:16 * i + 16, :],
                                  t[rlo:rlo + 16, csl])
                    eng.dma_start(sw[16 * i:16 * i + 8, :],
                                  t[rlo + 8:rlo + 16, csl])
                    eng.dma_start(sw[16 * i + 8:16 * i + 16, :],
                                  t[rlo:rlo + 8, csl])
                n = 16 * len(slices)
                nc.vector.tensor_mul(pk[:n, :], pk[:n, :], cos_t[:n, :])
                nc.vector.tensor_mul(sw[:n, :], sw[:n, :], sin_t[:n, :])
                nc.vector.tensor_add(pk[:n, :], pk[:n, :], sw[:n, :])
                for i, (t, rlo, clo) in enumerate(slices):
                    csl = slice(clo, clo + S)
                    eng = nc.sync if i % 2 == 0 else nc.scalar
                    eng.dma_start(t[rlo:rlo + 16, csl],
                                  pk[16 * i:16 * i + 16, :])

            # ---------- LN1 stats (row-major, col-tiled pairs) -------------
            def ln_stats(xpart, sqp, psp, rowp, part_name, want_col):
                """LN stats over the 128*KC feature dim, row-major.

                Per 512-block: sum -> stp[0:1] (tile_position (0,0)),
                sumsq -> stp[32:33] ((0,32)): concurrent col-tiled pairs,
                interleaved accumulation groups at different partitions of
                one PSUM bank. The raw sums are broadcast to all 128
                partitions with 1/D folded into the broadcast constant,
                and all stats math runs on full [128, S] tiles (keeps
                every DVE op partition-aligned). Returns (rstd_b, c_b).
                """
                row16 = rowp.tile([128, S], bf16, name=f"r16_{part_name}",
                                  tag="r16")
                for nb in range(NB):
                    nsl = slice(nb * NBLK, (nb + 1) * NBLK)
                    stp = psp.tile([128, NBLK], f32, name="stp")
                    for kc in range(KC):
                        sq_t = sqp.tile([128, NBLK], bf16, name="sq_t")
                        nc.scalar.activation(sq_t[:], xpart[:, kc, nsl],
                                             AF.Square)
                        nc.tensor.matmul(
                            stp[0:1, :], ones_c[:], xpart[:, kc, nsl],
                            start=(kc == 0), stop=(kc == KC - 1),
                            tile_position=(0, 0))
                        nc.tensor.matmul(
                            stp[32:33, :], ones_c[:], sq_t[:],
                            start=(kc == 0), stop=(kc == KC - 1),
                            tile_position=(0, 32))
                    nc.vector.tensor_copy(row16[0:1, nsl], stp[0:1, :])
                    nc.vector.tensor_copy(row16[32:33, nsl], stp[32:33, :])
                mean_b = rowp.tile([128, S], bf16, name="mean_b",
                                   tag="meanb")
                ex2_b = rowp.tile([128, S], bf16, name="ex2_b", tag="ex2b")
                for i in range(0, S, NBLK):
                    isl = slice(i, i + NBLK)
                    ps = psp.tile([128, NBLK], f32, name="bc_ps",
                                  tag="bc_ps", bufs=1)
                    nc.tensor.matmul(ps[:], invD128[0:1, :],
                                     row16[0:1, isl],
                                     start=True, stop=True)
                    nc.vector.tensor_copy(mean_b[:, isl], ps[:])
                    ps2 = psp.tile([128, NBLK], f32, name="bc_ps2",
                                   tag="bc_ps", bufs=1)
                    nc.tensor.matmul(ps2[:], invD128[32:33, :],
                                     row16[32:33, isl],
                                     start=True, stop=True)
                    nc.vector.tensor_copy(ex2_b[:, isl], ps2[:])
                m2_b = rowp.tile([128, S], bf16, name="m2_b", tag="m2b")
                nc.vector.tensor_mul(m2_b[:], mean_b[:], mean_b[:])
                nc.vector.tensor_sub(ex2_b[:], ex2_b[:], m2_b[:])
                rstd_b = rowp.tile([128, S], bf16, name="rstd_b",
                                   tag="rstdb")
                nc.scalar.activation(rstd_b[:], ex2_b[:], AF.Sqrt,
                                     bias=eps_c[:])
                with nc.allow_low_precision(reason="rstd in bf16 is fine"):
                    nc.vector.reciprocal(rstd_b[:], rstd_b[:])
                c_b = rowp.tile([128, S], bf16, name="c_b", tag="cb")
                nc.vector.tensor_mul(c_b[:], mean_b[:], rstd_b[:])
                if want_col:
                    for nm, row, col in (("rstd", rstd_b, rstd_col_mem),
                                         ("c", c_b, c_col_mem)):
                        dr = dramp.tile([S], bf16,
                                        name=f"st_{nm}_{part_name}")
                        nc.sync.dma_start(
                            dr[:].rearrange("(o a) -> o a", o=1),
                            row[0:1, :])
                        col16 = rowp.tile([128, LT], bf16,
                                          name=f"c16_{nm}", tag="col16")
                        nc.sync.dma_start(
                            col16[:],
                            dr[:].rearrange("(t p) -> p t", p=128))
                        nc.vector.tensor_copy(col[:], col16[:])
                return rstd_b, c_b

            def proj_raw(wt, dst, dst_off, xpart, psp):
                """Raw projection matmuls, copied to dst bf16 (no LN
                dependency: emitted before the stats so the PE never
                waits on the ScalarE Square stream)."""
                for m in range(2):
                    for nb in range(NB):
                        ps = psp.tile([128, NBLK], f32, name="proj_ps")
                        for kc in range(KC):
                            nc.tensor.matmul(
                                ps[:],
                                wt[:, kc, m * 128:(m + 1) * 128],
                                xpart[:, kc, nb * NBLK:(nb + 1) * NBLK],
                                start=(kc == 0), stop=(kc == KC - 1))
                        d = dst[m][:, dst_off + nb * NBLK:
                                   dst_off + (nb + 1) * NBLK]
                        nc.vector.tensor_copy(d, ps[:])

            def proj_epi(dst, dst_off, rstd_b, c_b, ws_t, b_t, scrp):
                """LN epilogue in place: d = d*rstd - (c*ws - bias)."""
                for m in range(2):
                    for nb in range(NB):
                        sl = slice(nb * NBLK, (nb + 1) * NBLK)
                        d = dst[m][:, dst_off + nb * NBLK:
                                   dst_off + (nb + 1) * NBLK]
                        cw = scrp.tile([128, NBLK], bf16, name="cw_nb",
                                       tag="cw_nb", bufs=2)
                        nc.vector.tensor_scalar(
                            out=cw[:], in0=c_b[:, sl],
                            scalar1=ws_t[:, m:m + 1],
                            scalar2=b_t[:, m:m + 1],
                            op0=Alu.mult, op1=Alu.subtract)
                        nc.vector.tensor_mul(d, d, rstd_b[:, sl])
                        nc.vector.tensor_sub(d, d, cw[:])

            with (
                tc.tile_pool(name="wqkv", bufs=1) as wqkvp,
                tc.tile_pool(name="psq", bufs=2, space="PSUM") as psqp,
                tc.tile_pool(name="psst", bufs=2, space="PSUM") as psstp,
            ):
                wq_t = wqkvp.tile([128, KC, J], bf16)
                wk_t = wqkvp.tile([128, KC, J], bf16)
                wv_t = wqkvp.tile([128, KC, J], bf16)

                # ----- phase 1a: memory part -----
                with (
                    tc.tile_pool(name="xm", bufs=1) as xmp,
                    tc.tile_pool(name="sqa", bufs=2) as sqap,
                    tc.tile_pool(name="rowa", bufs=1) as rowap,
                ):
                    xm_t = xmp.tile([128, KC, S], bf16)
                    nc.sync.dma_start(wk_t[:], wk_e[:])
                    for h in range(2):
                        hs = slice(h * S // 2, (h + 1) * S // 2)
                        for kc in range(KC):
                            eng = nc.sync if kc % 2 == 0 else nc.scalar
                            eng.dma_start(xm_t[:, kc, hs], xm_e[:, kc, hs])
                    nc.scalar.dma_start(wv_t[:], wv_e[:])
                    nc.sync.dma_start(wq_t[:], wq_e[:])
                    proj_raw(wk_t, kT, 0, xm_t, psqp)
                    rstd_bm, c_bm = ln_stats(
                        xm_t, sqap, psstp, rowap, "mem", want_col=True)
                    bcast_rows(wsvb, wsv_row, J, psqp, onesf)
                    bcast_rows(bvb, bv_row, J, psqp, onesf)
                    # v_mem row-major: lhsT = xm l-tile, rhs = wv
                    for lt in range(LT):
                        ps = psqp.tile([128, J], f32, name="vm_ps",
                                       bufs=2)
                        for kc in range(KC):
                            nc.tensor.matmul(
                                ps[:],
                                xm_t[:, kc, lt * 128:(lt + 1) * 128],
                                wv_t[:, kc, :],
                                start=(kc == 0), stop=(kc == KC - 1))
                        nc.vector.tensor_copy(v_mem[:, lt, :], ps[:])
                    proj_epi(kT, 0, rstd_bm, c_bm, wsk_t, bk_t, sqap)
                    for lt in range(LT):
                        cwv = sqap.tile([128, J], f32, name="cwv")
                        nc.vector.tensor_scalar(
                            out=cwv[:], in0=wsvb[:],
                            scalar1=c_col_mem[:, lt:lt + 1], scalar2=None,
                            op0=Alu.mult)
                        nc.vector.tensor_scalar_mul(
                            v_mem[:, lt, :], v_mem[:, lt, :],
                            rstd_col_mem[:, lt:lt + 1])
                        nc.vector.tensor_sub(
                            v_mem[:, lt, :], v_mem[:, lt, :], cwv[:])
                        nc.vector.tensor_add(
                            v_mem[:, lt, :], v_mem[:, lt, :], bvb[:])

                # ----- phase 1b: hidden part -----
                with (
                    tc.tile_pool(name="xh", bufs=1) as xhp,
                    tc.tile_pool(name="sqb", bufs=2) as sqbp,
                    tc.tile_pool(name="rowb", bufs=1) as rowbp,
                ):
                    xh_t = xhp.tile([128, KC, S], bf16)
                    for h in range(2):
                        hs = slice(h * S // 2, (h + 1) * S // 2)
                        for kc in range(KC):
                            eng = nc.sync if kc % 2 == 0 else nc.scalar
                            eng.dma_start(xh_t[:, kc, hs], xh_e[:, kc, hs])
                    proj_raw(wq_t, qT, 0, xh_t, psqp)
                    rstd_bh, c_bh = ln_stats(
                        xh_t, sqbp, psstp, rowbp, "hid", want_col=False)
                    proj_raw(wk_t, kT, S, xh_t, psqp)
                    proj_epi(qT, 0, rstd_bh, c_bh, wsq_t, bq_t, sqbp)
                    # packed RoPE: q + k-mem (8 groups = 128 partitions),
                    # then k-hid (4 groups), overlapping the v_h matmuls
                    load_rope_tables()
                    rope_packed(
                        [(qT[0], 0, 0), (qT[0], 64, 0),
                         (qT[1], 0, 0), (qT[1], 64, 0),
                         (kT[0], 0, 0), (kT[0], 64, 0),
                         (kT[1], 0, 0), (kT[1], 64, 0)], "a")
                    proj_raw(wv_t, vTh, 0, xh_t, psqp)
                    proj_epi(kT, S, rstd_bh, c_bh, wsk_t, bk_t, sqbp)
                    rope_packed(
                        [(kT[0], 0, S), (kT[0], 64, S),
                         (kT[1], 0, S), (kT[1], 64, S)], "b")
                    proj_epi(vTh, 0, rstd_bh, c_bh, wsvc_t, bvc_t, sqbp)
            ropep.release()

            # ---------- loop 1: attention + local o_p + AllReduce ---------
            op_dram = [dramp.tile([D, (hi - lo) * NBLK], bf16,
                                  name=f"op_d{g}")
                       for g, (lo, hi) in enumerate(ARG)]
            rs_sh = [dramp.tile([J, (hi - lo) * NBLK], bf16,
                                name=f"rs_sh{g}")
                     for g, (lo, hi) in enumerate(ARG)]
            h_sh = [dshp.tile([D, (hi - lo) * NBLK], bf16, name=f"h_sh{g}",
                              addr_space="Shared")
                    for g, (lo, hi) in enumerate(ARG)]
            with (
                tc.tile_pool(name="maskp", bufs=1) as maskp,
                tc.tile_pool(name="wop", bufs=1) as wop,
                tc.tile_pool(name="attw", bufs=12) as attwp,
                tc.tile_pool(name="attt", bufs=4) as atttp,
                tc.tile_pool(name="cmbp", bufs=3) as cmbp,
                tc.tile_pool(name="attr", bufs=2) as attrp,
                tc.tile_pool(name="oc", bufs=3) as ocp,
                tc.tile_pool(name="psS", bufs=3, space="PSUM") as psSp,
                tc.tile_pool(name="psA", bufs=1, space="PSUM") as psAp,
                tc.tile_pool(name="psD", bufs=1, space="PSUM") as psDp,
            ):
                masks_t = maskp.tile([128, 4, NBLK], bf16)
                nc.sync.dma_start(masks_t[:], masks_e[:])
                wo_t = wop.tile([128, MD, D], bf16)
                nc.sync.dma_start(wo_t[:], wo_e[:])
                for b in range(NB):
                    bsl = slice(b * NBLK, (b + 1) * NBLK)
                    T = 4 * b + 4
                    den4 = psDp.tile([128, NBLK], f32, name="den4")
                    sf4 = psSp.tile([128, NBLK], f32, name="sf4",
                                    bufs=1)
                    swf4 = attrp.tile([128, NBLK], f32, name="swf4")
                    dent4 = attrp.tile([128, NBLK], f32, name="dent4")
                    rcp4 = attrp.tile([128, NBLK], f32, name="rcp4")
                    swb4 = attrp.tile([128, NBLK], bf16, name="swb4")
                    rcpb4 = attrp.tile([128, NBLK], bf16, name="rcpb4")
                    ap_ps = [psAp.tile([128, NBLK], f32, name=f"ap{m}",
                                       bufs=1)
                             for m in range(2)]
                    # self-key q.k products hoisted: they only need the
                    # rope'd q/k, so the DVE does them while the PE runs
                    # the score matmuls
                    qks = []
                    for m in range(2):
                        for io, o in enumerate((0, 64)):
                            hsl = slice(o, o + 64)
                            qk = atttp.tile([128, NBLK], bf16,
                                            name=f"qk{2 * m + io}",
                                            tag=f"qk{2 * m + io}", bufs=2)
                            nc.vector.tensor_mul(
                                qk[hsl, :], qT[m][hsl, bsl],
                                kT[m][hsl,
                                      S + b * NBLK:S + (b + 1) * NBLK])
                            qks.append(qk)
                    # self-key column sums + exp: independent of the key
                    # loop (sf4 has its own PSUM bank, so no ring wait)
                    heads = [(m, io, o) for m in range(2)
                             for io, o in enumerate((0, 64))]
                    for m, io, o in heads:
                        hsl = slice(o, o + 64)
                        r = 32 * (2 * m + io)
                        nc.tensor.matmul(
                            sf4[r:r + 1, :], ones_c[hsl, 0:1],
                            qks[2 * m + io][hsl, :],
                            start=True, stop=True,
                            tile_position=(o, r))
                    for m, io, o in heads:
                        rsl = slice(32 * (2 * m + io),
                                    32 * (2 * m + io) + 1)
                        nc.scalar.activation(
                            swf4[rsl, :], sf4[rsl, :], AF.Exp,
                            scale=0.125)
                        nc.vector.tensor_copy(swb4[rsl, :], swf4[rsl, :])

                    def q_lo(t):
                        """First unmasked q column for key-tile t (the
                        diagonal band is strictly causal: keys t*128+ii
                        only reach q > t*128+ii within the block)."""
                        return max(0, (t - 4 * b) * 128)

                    def s_pair(t):
                        """Score MMs for key-tile t, all 4 heads: two
                        row-tiled concurrent pairs, band-narrowed."""
                        tsl = slice(t * 128, (t + 1) * 128)
                        c0 = q_lo(t)
                        ss = []
                        for m in range(2):
                            for o in (0, 64):
                                hsl = slice(o, o + 64)
                                s_ps = psSp.tile([128, NBLK], f32,
                                                 name="s_ps")
                                nc.tensor.matmul(
                                    s_ps[:, c0:], kT[m][hsl, tsl],
                                    qT[m][hsl,
                                          b * NBLK + c0:(b + 1) * NBLK],
                                    start=True, stop=True,
                                    tile_position=(o, 0))
                                ss.append(s_ps)
                        return ss

                    ss_cur = s_pair(0)
                    for t in range(T):
                        ss_next = s_pair(t + 1) if t + 1 < T else None
                        c0 = q_lo(t)
                        # exp (+ mask on the diagonal band) on ScalarE/DVE
                        ws = []
                        for i, (m, o) in enumerate(
                                ((0, 0), (0, 64), (1, 0), (1, 64))):
                            w_t = attwp.tile([128, NBLK], bf16, name="w_t")
                            nc.scalar.activation(
                                w_t[:, c0:], ss_cur[i][:, c0:], AF.Exp,
                                scale=0.125)
                            if t >= 4 * b:
                                nc.vector.tensor_mul(
                                    w_t[:, c0:], w_t[:, c0:],
                                    masks_t[:, t - 4 * b, c0:])
                            ws.append(w_t)
                        # AV: col-tiled concurrent pairs per m
                        for m in range(2):
                            for io, o in enumerate((0, 64)):
                                nc.tensor.matmul(
                                    ap_ps[m][o:o + 64, c0:],
                                    v_mem[:, t,
                                          m * 128 + o:m * 128 + o + 64],
                                    ws[2 * m + io][:, c0:],
                                    start=(t == 0), stop=(t == T - 1),
                                    tile_position=(0, o))
                        # denominators: concurrent pairs at cols r
                        for m in range(2):
                            for io, o in enumerate((0, 64)):
                                r = 32 * (2 * m + io)
                                nc.tensor.matmul(
                                    den4[r:r + 1, c0:], ones_c[:, 0:1],
                                    ws[2 * m + io][:, c0:],
                                    start=(t == 0), stop=(t == T - 1),
                                    tile_position=(0, r))
                        ss_cur = ss_next

                    # self-key epilogue: only the den-dependent part
                    for m, io, o in heads:
                        rsl = slice(32 * (2 * m + io),
                                    32 * (2 * m + io) + 1)
                        nc.vector.tensor_add(
                            dent4[rsl, :], den4[rsl, :], swf4[rsl, :])
                        nc.vector.reciprocal(rcp4[rsl, :], dent4[rsl, :])
                        nc.vector.tensor_copy(rcpb4[rsl, :], rcp4[rsl, :])
                    # broadcast self_w and 1/den to each head's 64 rows
                    for m in range(2):
                        sb_ps = psSp.tile([128, NBLK], f32, name="sb_ps",
                                          tag="sbrb", bufs=1)
                        rb_ps = psSp.tile([128, NBLK], f32, name="rb_ps",
                                          tag="sbrb", bufs=1)
                        for io, o in enumerate((0, 64)):
                            r = 32 * (2 * m + io)
                            rsl = slice(r, r + 1)
                            nc.tensor.matmul(
                                sb_ps[o:o + 64, :], ones128[rsl, 0:64],
                                swb4[rsl, :], start=True, stop=True,
                                tile_position=(r, o))
                            nc.tensor.matmul(
                                rb_ps[o:o + 64, :], ones128[rsl, 0:64],
                                rcpb4[rsl, :], start=True, stop=True,
                                tile_position=(r, o))
                        # combine: (attn + self_w * vTh) / den
                        t0 = atttp.tile([128, NBLK], bf16, name="t0")
                        nc.vector.tensor_mul(t0[:], vTh[m][:, bsl],
                                             sb_ps[:])
                        t1 = atttp.tile([128, NBLK], bf16, name="t1")
                        nc.vector.tensor_add(t1[:], ap_ps[m][:], t0[:])
                        cmb = cmbp.tile([128, NBLK], bf16, name=f"cmb{m}")
                        nc.vector.tensor_mul(cmb[:], t1[:], rb_ps[:])
                        if m == 0:
                            cmb0 = cmb
                        else:
                            cmb1 = cmb

                    # local o_p: full-D partial from this core's heads;
                    # stored via ONE DMA per block so the collective
                    # doorbell has a single completion to wait on
                    g, lo, hi = argrp(b)
                    csl = slice((b - lo) * NBLK, (b - lo + 1) * NBLK)
                    oc_blk = ocp.tile([128, KC, NBLK], bf16,
                                      name="oc_blk", bufs=2)
                    for md16 in range(D // 128):
                        ps = psSp.tile([128, NBLK], f32, name="o_ps",
                                       tag="s_ps")
                        nc.tensor.matmul(
                            ps[:], wo_t[:, 0, md16 * 128:(md16 + 1) * 128],
                            cmb0[:], start=True, stop=False)
                        nc.tensor.matmul(
                            ps[:], wo_t[:, 1, md16 * 128:(md16 + 1) * 128],
                            cmb1[:], start=False, stop=True)
                        nc.scalar.copy(oc_blk[:, md16, :], ps[:])
                    nc.scalar.dma_start(
                        op_dram[g][:, csl].rearrange(
                            "(t p) s -> p t s", p=128),
                        oc_blk[:])
                    if b == hi - 1:
                        # ReduceScatter gives this core's own jsl rows of
                        # the o-sum (residual path); AllGather of that is
                        # the full-D sum (LN2 path).
                        nc.gpsimd.collective_compute(
                            "ReduceScatter", mybir.AluOpType.add,
                            replica_groups=rg,
                            ins=[op_dram[g].opt()], outs=[rs_sh[g].opt()])
                        nc.gpsimd.collective_compute(
                            "AllGather", mybir.AluOpType.bypass,
                            replica_groups=rg,
                            ins=[rs_sh[g].opt()], outs=[h_sh[g].opt()])
            statkp.release()
            kqvp.release()

            # MLP weights: DMAs are emitted inside loop 2 after block 0's
            # h/rs loads so they don't delay the LN2 critical path
            wgup = tc.alloc_tile_pool(name="wgu", bufs=1)
            wdp = tc.alloc_tile_pool(name="wd", bufs=1)
            wg_t = wgup.tile([128, KC, FFL], bf16)
            wu_t = wgup.tile([128, KC, FFL], bf16)
            wd_t = wdp.tile([128, FFL // 128, D], bf16)

            # ---------- loop 2: LN2 + gated MLP + down + out --------------
            # row-parallel down: each core contracts its own FFL slice of
            # m (straight from SBUF) into a full-D partial, which is
            # ReduceScattered; out rows jsl come back summed.
            dp_dram = [dramp.tile([D, (hi - lo) * NBLK], bf16,
                                  name=f"dp_d{g}")
                       for g, (lo, hi) in enumerate(DRG)]
            dp_rs = [dramp.tile([J, (hi - lo) * NBLK], bf16,
                                name=f"dp_rs{g}")
                     for g, (lo, hi) in enumerate(DRG)]
            res_pool = tc.alloc_tile_pool(name="res", bufs=1)
            res_t = res_pool.tile([128, MD, S], bf16)
            for md in range(MD):
                nc.scalar.dma_start(res_t[:, md, :], xhres_e[:, md, :])
            with (
                tc.tile_pool(name="hblk", bufs=2) as hblkp,
                tc.tile_pool(name="xh2", bufs=2) as xh2p,
                tc.tile_pool(name="sq5", bufs=2) as sq5p,
                tc.tile_pool(name="sm5", bufs=1) as sm5p,
                tc.tile_pool(name="mloc", bufs=2) as mlocp,
                tc.tile_pool(name="gut", bufs=2) as gutp,
                tc.tile_pool(name="dcp", bufs=2) as dcp,
                tc.tile_pool(name="outt", bufs=2) as outtp,
                tc.tile_pool(name="psG", bufs=2, space="PSUM") as psGp,
                tc.tile_pool(name="psU", bufs=2, space="PSUM") as psUp,
                tc.tile_pool(name="psst5", bufs=1, space="PSUM") as psst5p,
                tc.tile_pool(name="psDn", bufs=1, space="PSUM") as psDnp,
            ):
                def down_block(b, m_loc):
                    """Row-parallel down: full-D partial from this core's
                    FFL slice of m, straight from SBUF; ReduceScatter
                    sums across cores and hands back the jsl rows."""
                    g, lo, hi = drgrp(b)
                    csl = slice((b - lo) * NBLK, (b - lo + 1) * NBLK)
                    for q in range(4):
                        dcq = dcp.tile([128, 4, NBLK], bf16, name="dcq")
                        for sub in range(4):
                            md16 = q * 4 + sub
                            ps = psDnp.tile([128, NBLK], f32, name="d_ps",
                                            bufs=2)
                            for fc in range(FFL // 128):
                                nc.tensor.matmul(
                                    ps[:],
                                    wd_t[:, fc,
                                         md16 * 128:(md16 + 1) * 128],
                                    m_loc[:, fc, :],
                                    start=(fc == 0),
                                    stop=(fc == FFL // 128 - 1))
                            nc.scalar.copy(dcq[:, sub, :], ps[:])
                        eng = nc.sync if q % 2 == 0 else nc.scalar
                        eng.dma_start(
                            dp_dram[g][q * 512:(q + 1) * 512,
                                       csl].rearrange(
                                "(t p) s -> p t s", p=128),
                            dcq[:])
                    if b == hi - 1:
                        nc.gpsimd.collective_compute(
                            "ReduceScatter", mybir.AluOpType.add,
                            replica_groups=rg,
                            ins=[dp_dram[g].opt()], outs=[dp_rs[g].opt()])

                def ln2_block(b):
                    """h = xh + o_sum, residual update, LN2 stats +
                    normalize. Returns the h2 block tile."""
                    bsl = slice(b * NBLK, (b + 1) * NBLK)
                    ag, alo, ahi = argrp(b)
                    acsl = slice((b - alo) * NBLK, (b - alo + 1) * NBLK)
                    xhb = xh2p.tile([128, KC, NBLK], bf16, name="xhb",
                                    bufs=1)
                    nc.scalar.dma_start(xhb[:], xh_e[:, :, bsl])
                    h_t = hblkp.tile([128, KC, NBLK], bf16, name="h_t")
                    for tq in range(4):
                        nc.sync.dma_start(
                            h_t[:, tq * 4:(tq + 1) * 4, :],
                            h_sh[ag][tq * 512:(tq + 1) * 512,
                                     acsl].rearrange(
                                "(t p) s -> p t s", p=128))
                    for kc in range(KC):
                        nc.vector.tensor_add(
                            h_t[:, kc, :], h_t[:, kc, :], xhb[:, kc, :])
                    # residual: own jsl rows of the o-sum from the
                    # ReduceScatter output, added onto the xh_res preload
                    rs_b = xh2p.tile([128, MD, NBLK], bf16, name="rs_b")
                    nc.sync.dma_start(
                        rs_b[:], rs_sh[ag][:, acsl].rearrange(
                            "(t p) s -> p t s", p=128))
                    for md in range(MD):
                        nc.vector.tensor_add(
                            res_t[:, md, bsl], res_t[:, md, bsl],
                            rs_b[:, md, :])
                    # LN2 stats: col-tiled (sum, sumsq) pair
                    stp = psst5p.tile([128, NBLK], f32, name="st5")
                    for kc in range(KC):
                        sq_t = sq5p.tile([128, NBLK], bf16, name="sq5_t")
                        nc.scalar.activation(sq_t[:], h_t[:, kc, :],
                                             AF.Square)
                        nc.tensor.matmul(
                            stp[0:1, :], ones_c[:], h_t[:, kc, :],
                            start=(kc == 0), stop=(kc == KC - 1),
                            tile_position=(0, 0))
                        nc.tensor.matmul(
                            stp[32:33, :], ones_c[:], sq_t[:],
                            start=(kc == 0), stop=(kc == KC - 1),
                            tile_position=(0, 32))
                    r16 = sm5p.tile([128, NBLK], bf16, name="r165",
                                    tag="sm5r", bufs=1)
                    nc.vector.tensor_copy(r16[0:1, :], stp[0:1, :])
                    nc.vector.tensor_copy(r16[32:33, :], stp[32:33, :])
                    mean_b = sm5p.tile([128, NBLK], bf16, name="mean5b",
                                       tag="sm5m", bufs=1)
                    ex2_b = sm5p.tile([128, NBLK], bf16, name="ex25b",
                                      tag="sm5e", bufs=1)
                    ps = psst5p.tile([128, NBLK], f32, name="bc_ps",
                                     tag="bc_ps", bufs=1)
                    nc.tensor.matmul(ps[:], invD128[0:1, :], r16[0:1, :],
                                     start=True, stop=True)
                    nc.vector.tensor_copy(mean_b[:], ps[:])
                    ps2 = psst5p.tile([128, NBLK], f32, name="bc_ps2",
                                      tag="bc_ps", bufs=1)
                    nc.tensor.matmul(ps2[:], invD128[32:33, :],
                                     r16[32:33, :],
                                     start=True, stop=True)
                    nc.vector.tensor_copy(ex2_b[:], ps2[:])
                    m2_b = sm5p.tile([128, NBLK], bf16, name="m25b",
                                     tag="sm5m2", bufs=1)
                    nc.vector.tensor_mul(m2_b[:], mean_b[:], mean_b[:])
                    nc.vector.tensor_sub(ex2_b[:], ex2_b[:], m2_b[:])
                    rstd_b = sm5p.tile([128, NBLK], bf16, name="rstd5b",
                                       tag="sm5rs", bufs=1)
                    nc.scalar.activation(rstd_b[:], ex2_b[:], AF.Sqrt,
                                         bias=eps_c[:])
                    with nc.allow_low_precision(
                            reason="rstd in bf16 is fine"):
                        nc.vector.reciprocal(rstd_b[:], rstd_b[:])
                    # normalize in place: h_t becomes h2
                    for kc in range(KC):
                        hm = sq5p.tile([128, NBLK], bf16, name="hm5")
                        nc.vector.tensor_sub(hm[:], h_t[:, kc, :],
                                             mean_b[:])
                        nc.vector.tensor_mul(h_t[:, kc, :], hm[:],
                                             rstd_b[:])
                    return h_t

                def gu_block(b, h2_t):
                    m_loc = mlocp.tile([128, FFL // 128, NBLK], bf16,
                                       name="m_loc")
                    for mf in range(FFL // 128):
                        psg = psGp.tile([128, NBLK], f32, name="g_ps")
                        psu = psUp.tile([128, NBLK], f32, name="u_ps")
                        for kc in range(KC):
                            nc.tensor.matmul(
                                psg[:],
                                wg_t[:, kc, mf * 128:(mf + 1) * 128],
                                h2_t[:, kc, :],
                                start=(kc == 0), stop=(kc == KC - 1))
                            nc.tensor.matmul(
                                psu[:],
                                wu_t[:, kc, mf * 128:(mf + 1) * 128],
                                h2_t[:, kc, :],
                                start=(kc == 0), stop=(kc == KC - 1))
                        sg = gutp.tile([128, NBLK], bf16, name="sg")
                        nc.scalar.activation(
                            sg[:], psg[:], AF.Sigmoid,
                            bias=bg_t[:, mf:mf + 1])
                        silu = gutp.tile([128, NBLK], bf16, name="silu")
                        nc.vector.scalar_tensor_tensor(
                            out=silu[:], in0=psg[:],
                            scalar=bg_t[:, mf:mf + 1], in1=sg[:],
                            op0=Alu.add, op1=Alu.mult)
                        nc.vector.scalar_tensor_tensor(
                            out=m_loc[:, mf, :], in0=psu[:],
                            scalar=bu_t[:, mf:mf + 1], in1=silu[:],
                            op0=Alu.add, op1=Alu.mult)
                    return m_loc

                # MLP weight loads: the sync queue is drained by the end
                # of attention, so these issue during the o-sum chains
                nc.sync.dma_start(wg_t[:], wg_e[:])
                nc.scalar.dma_start(wu_t[:], wu_e[:])
                nc.sync.dma_start(wd_t[:], wd_e[:])
                def out_block(b):
                    """Own jsl rows of the down-sum + residual -> out."""
                    bsl = slice(b * NBLK, (b + 1) * NBLK)
                    g, lo, hi = drgrp(b)
                    csl = slice((b - lo) * NBLK, (b - lo + 1) * NBLK)
                    rsd = xh2p.tile([128, MD, NBLK], bf16, name="rsd")
                    nc.sync.dma_start(
                        rsd[:], dp_rs[g][:, csl].rearrange(
                            "(t p) s -> p t s", p=128))
                    for md in range(MD):
                        ot = outtp.tile([128, NBLK], f32, name="ot")
                        nc.vector.tensor_add(
                            ot[:], res_t[:, md, bsl], rsd[:, md, :])
                        nc.sync.dma_start(out_e[md, :, bsl], ot[:])

                # software pipeline: gu(b-1)+down(b-1) fill the PE while
                # block b's LN2 vector chain runs; group-0 outputs are
                # interleaved so only out(2,3) remain in the tail
                h2_prev = None
                for b in range(NB):
                    if h2_prev is not None:
                        ml = gu_block(b - 1, h2_prev)
                        down_block(b - 1, ml)
                        if b - 1 == 2:
                            out_block(0)
                            out_block(1)
                    h2_prev = ln2_block(b)
                ml = gu_block(NB - 1, h2_prev)
                down_block(NB - 1, ml)
                out_block(2)
                out_block(3)
            res_pool.release()
            wdp.release()
            wgup.release()

    return nc


# ---------------------------------------------------------------------------
# Host side
# ---------------------------------------------------------------------------

def _chunkT(a):
    """[R, D] -> [128, D//128, R] view for lhsT/rhs chunk layout.

    Result[p, kc, r] = a[r, kc*128 + p].
    """
    R, Dd = a.shape
    return np.ascontiguousarray(
        a.reshape(R, Dd // 128, 128).transpose(2, 1, 0))


def prepare_inputs(hidden_states, memory, position_ids,
                   ln1_w, ln1_b, ln2_w, ln2_b,
                   Wq, Wk, Wv, Wo, Wg, Wu, Wd, S):
    """Build the 8 per-core in_maps (numpy host prep)."""
    f32 = np.float32
    hid = np.asarray(hidden_states, f32)[0]       # [S, D]
    mem = np.asarray(memory, f32)[0]
    pos = np.asarray(position_ids)[0].astype(np.float64)

    Wq1 = np.asarray(Wq, f32) * np.asarray(ln1_w, f32)[None, :]
    Wk1 = np.asarray(Wk, f32) * np.asarray(ln1_w, f32)[None, :]
    Wv1 = np.asarray(Wv, f32) * np.asarray(ln1_w, f32)[None, :]
    bq = np.asarray(Wq, f32) @ np.asarray(ln1_b, f32)
    bk = np.asarray(Wk, f32) @ np.asarray(ln1_b, f32)
    bv = np.asarray(Wv, f32) @ np.asarray(ln1_b, f32)
    Wg2 = np.asarray(Wg, f32) * np.asarray(ln2_w, f32)[None, :]
    Wu2 = np.asarray(Wu, f32) * np.asarray(ln2_w, f32)[None, :]
    bg = np.asarray(Wg, f32) @ np.asarray(ln2_b, f32)
    bu = np.asarray(Wu, f32) @ np.asarray(ln2_b, f32)
    Wo_ = np.asarray(Wo, f32)
    Wd_ = np.asarray(Wd, f32)

    # x^T chunk layouts (shared by all cores)
    xm = _chunkT(mem).astype(BF16)                # [128, KC, S]
    xh = _chunkT(hid).astype(BF16)

    # rope tables [128, 2S], row pattern period 16
    inv = BASE ** (-(np.arange(8, dtype=np.float64) * 2) / RD)
    t = pos[:, None] * inv[None, :]               # [S, 8]
    cos8 = np.cos(t).T                            # [8, S]
    sin8 = np.sin(t).T
    cos16 = np.concatenate([cos8, cos8], 0)       # [16, S]
    sin16 = np.concatenate([-sin8, sin8], 0)
    cosf = np.tile(np.concatenate([cos16, cos16], 1), (8, 1))  # [128, 2S]
    sinf = np.tile(np.concatenate([sin16, sin16], 1), (8, 1))
    cosf = cosf.astype(BF16)
    sinf = sinf.astype(BF16)

    # strict-causal masks for the 4 diagonal-band offsets
    ii = np.arange(128)[:, None]
    jj = np.arange(NBLK)[None, :]
    masks = np.stack(
        [(ii + 128 * o < jj) for o in range(4)], 1).astype(BF16)  # [128,4,512]

    in_maps = []
    for c in range(N_CORES):
        jsl = slice(c * J, (c + 1) * J)
        fsl = slice(c * FFL, (c + 1) * FFL)
        wq_c = Wq1[jsl]                            # [J, D]
        wk_c = Wk1[jsl]
        wv_c = Wv1[jsl]
        im = {
            "xm": xm, "xh": xh,
            "xh_res": np.ascontiguousarray(
                hid[:, c * J:(c + 1) * J].T.reshape(MD, 128, S)
                .transpose(1, 0, 2)).astype(BF16),
            "wq": _chunkT(wq_c).astype(BF16),
            "wk": _chunkT(wk_c).astype(BF16),
            "wv": _chunkT(wv_c).astype(BF16),
            "wo_p": _chunkT(Wo_[:, jsl]).astype(BF16),
            "wg": _chunkT(Wg2[fsl]).astype(BF16),
            "wu": _chunkT(Wu2[fsl]).astype(BF16),
            "wd": _chunkT(Wd_[:, fsl]).astype(BF16),
            "wsq": np.ascontiguousarray(
                wq_c.sum(1).reshape(MD, 128).T).astype(f32),
            "wsk": np.ascontiguousarray(
                wk_c.sum(1).reshape(MD, 128).T).astype(f32),
            "wsvc": np.ascontiguousarray(
                wv_c.sum(1).reshape(MD, 128).T).astype(f32),
            "bq": np.ascontiguousarray(
                bq[jsl].reshape(MD, 128).T).astype(f32),
            "bk": np.ascontiguousarray(
                bk[jsl].reshape(MD, 128).T).astype(f32),
            "bvc": np.ascontiguousarray(
                bv[jsl].reshape(MD, 128).T).astype(f32),
            "wsv_row": wv_c.sum(1)[None, :].astype(f32),
            "bv_row": bv[jsl][None, :].astype(f32),
            "bg": np.ascontiguousarray(
                bg[fsl].reshape(FFL // 128, 128).T).astype(f32),
            "bu": np.ascontiguousarray(
                bu[fsl].reshape(FFL // 128, 128).T).astype(f32),
            "rope_cos": cosf, "rope_sinsg": sinf,
            "masks": masks,
        }
        in_maps.append(im)
    return in_maps


def assemble_output(results, S):
    outT = np.concatenate(
        [np.asarray(results[c]["out"]).reshape(J, S)
         for c in range(N_CORES)], 0)              # [D, S]
    return np.ascontiguousarray(outT.T).reshape(1, S, D).astype(np.float32)


_GRAPH_CACHE = {}


def get_graph(S):
    if S not in _GRAPH_CACHE:
        _GRAPH_CACHE[S] = build_graph(S)
    return _GRAPH_CACHE[S]


def kernel(hidden_states, memory, attention_mask, position_ids,
           ln1_w, ln1_b, ln2_w, ln2_b, Wq, Wk, Wv, Wo, Wg, Wu, Wd):
    from concourse.bass_utils import run_bass_kernel_spmd

    S = np.asarray(hidden_states).shape[1]
    in_maps = prepare_inputs(
        hidden_states, memory, position_ids, ln1_w, ln1_b, ln2_w, ln2_b,
        Wq, Wk, Wv, Wo, Wg, Wu, Wd, S)
    nc = get_graph(S)
    res = run_bass_kernel_spmd(nc, in_maps, core_ids=list(range(N_CORES)))
    return assemble_output(res.results, S)
